# revision 74
# baseline (speedup 1.0000x reference)
"""Trainium2 Bass kernel for nn_MultiHeadDilatedState.

Sharding: data-parallel over batch (B=8 -> 8 cores, one sequence per core).
Weights replicated. Per-core dataflow is channel-major [768, 4096], with the
whole h pipeline held in fp16 SBUF:

  x ships from the host BOTH pre-transposed fp16 [128, chunk, S] and as an
  fp8-e4m3 DoubleRow-interleaved copy [128, pair, 2, S]. A short warmup
  matmul spin keeps the PE's HAM clock warm while the first x block lands.
  Phase A: router fp16; GLU gate half as fp8 DoubleRow matmuls (K=256 per
  pass, sigmoid damps the quantization error), value half fp16; PSUM
  accumulates fp32. The router head-weights are broadcast to a
  [128, NC, S] per-channel replica via a DRAM-roundtrip DMA
  (partition-broadcast access pattern) for the stage-3 gating.
  Phase B (convs): per-head depthwise dilated conv = fp16 diagonal matmuls
  with column-shifted rhs APs; TWO chunks (4 heads) run concurrently using
  all four (row,col) 64x64 blocks of the PE array via tile_position; the
  residual folds into the tap-0 diag (1+w). Chunk-pair-outer/stage-inner
  ordering lets each pair's fp8 h casts (mix-gate input) overlap the next
  pair's conv. Stage-3 evacs fuse the head-weight gating (Scalar/Vector
  split).
  Phase C: mix-gate as fp8 DoubleRow matmuls off the cast h8; phase D
  keeps the mixing weights stationary and emits the output channel-major
  [768, S] (host transposes on unshard); C runs two s-tiles ahead of D,
  both descending to chase the conv's completion order.
"""

import os
import numpy as np

import concourse.bass as bass
import concourse.bacc as bacc
import concourse.mybir as mybir
import concourse.tile as tile
from concourse.bass_utils import run_bass_kernel_spmd

B, S, HID = 8, 4096, 768
NH, HD, KT = 12, 64, 4  # heads, head_dim, kernel taps
NC = 6                  # 768 / 128 channel chunks
NP = 3                  # chunk pairs
ST = 512                # token tile
NST = S // ST           # 8
F32 = mybir.dt.float32
F16 = mybir.dt.float16
F8 = mybir.dt.float8e4
DR = mybir.MatmulPerfMode.DoubleRow
SIG = mybir.ActivationFunctionType.Sigmoid
ADD = mybir.AluOpType.add
MUL = mybir.AluOpType.mult

DILATIONS = [(1, 2, 4), (1, 1, 1), (4, 8, 16), (8, 16, 32), (32, 64, 128),
             (64, 128, 256), (256, 512, 1024), (1, 100, 200), (1, 500, 1000),
             (1, 1024, 2048), (3, 9, 27), (5, 25, 125)]

# odd-chunk storage layout entering stage j: 1 = halves swapped. Each conv
# stage flips it (cross tile_position blocks exchange halves); GLU writes
# odd chunks swapped so the final stage lands back on normal order.
LIN = [1, 0, 1]


def build_bass():
    nc = bacc.Bacc()

    x_d = nc.dram_tensor("xbT", [128, NC, S], F16, kind="ExternalInput")
    x8_d = nc.dram_tensor("xb8", [128, NP, 2, S], F8, kind="ExternalInput")
    gvT_d = nc.dram_tensor("gvT16", [128, NC, HID], F16, kind="ExternalInput")
    gg8_d = nc.dram_tensor("gg8", [128, NP, 2, HID], F8, kind="ExternalInput")
    rwr_d = nc.dram_tensor("rwr16", [128, NC, 64], F16, kind="ExternalInput")
    rb_d = nc.dram_tensor("rb", [NH, 1], F32, kind="ExternalInput")
    convdiag_d = nc.dram_tensor("convdiag", [128, 9, 512], F16, kind="ExternalInput")
    convbias_d = nc.dram_tensor("convbias", [128, 3, 8], F32, kind="ExternalInput")
    mgw_d = nc.dram_tensor("mgw8", [128, NP, 2, HID], F8, kind="ExternalInput")
    mgb_d = nc.dram_tensor("mgb", [128, 32], F32, kind="ExternalInput")
    mixt_d = nc.dram_tensor("mixt16", [128, NC, HID], F16, kind="ExternalInput")
    mixbias_d = nc.dram_tensor("mixbias", [128, 32], F32, kind="ExternalInput")
    out_d = nc.dram_tensor("out", [HID, S], F16, kind="ExternalOutput")
    dbg_d = nc.dram_tensor("dbg", [NC, 128, S], F16, kind="ExternalOutput") if os.environ.get("KDBG") else None

    with tile.TileContext(nc) as tc:
        _body(tc, x_d, x8_d, gvT_d, gg8_d, rwr_d, rb_d, convdiag_d,
              convbias_d, mgw_d, mgb_d, mixt_d, mixbias_d, out_d, dbg_d)
    nc.finalize()
    return nc


def _conv_groups(j, P):
    """The four 64x64 PE-array blocks for chunk pair (2P, 2P+1) at stage j.

    Returns (chunk, ab, lhs_p0, out_p0, tile_position, dilation) per block.
    lhs_p0: partition base of the stationary diag (= rhs row group).
    out_p0: psum partition base (= col group).
    """
    cA, cB = 2 * P, 2 * P + 1
    hT = 2 * cB + 1 if LIN[j] else 2 * cB          # cB data on partitions 0:64
    hU = 2 * cB if LIN[j] else 2 * cB + 1          # cB data on partitions 64:128
    return [
        (cA, 0, 0, 0, (0, 0), DILATIONS[2 * cA][j]),
        (cA, 0, 64, 64, (64, 64), DILATIONS[2 * cA + 1][j]),
        (cB, 1, 0, 64, (0, 64), DILATIONS[hT][j]),
        (cB, 1, 64, 0, (64, 0), DILATIONS[hU][j]),
    ]


def _body(tc, x_d, x8_d, gvT_d, gg8_d, rwr_d, rb_d, convdiag_d, convbias_d,
          mgw_d, mgb_d, mixt_d, mixbias_d, out_d, dbg_d=None):
    nc = tc.nc

    with (
        tc.tile_pool(name="persist", bufs=1) as persist,
        tc.tile_pool(name="sig", bufs=4) as p_sig,
        tc.tile_pool(name="outsb", bufs=4) as p_out,
        tc.tile_pool(name="dscr", bufs=1, space="DRAM") as p_dram,
    ):
        # ---- persistent weights. Weight DMAs go on the Activation DGE
        # ring so the x loads (Sync ring) aren't stuck behind them;
        # earliest-needed weights first. ----
        rwr = persist.tile([128, NC, 64], F16, tag="rwr")
        rb_p = persist.tile([NH, 32], F32, tag="rb")
        rb = rb_p[:, 0:1]
        gvT = persist.tile([128, NC, HID], F16, tag="gvT")
        gg8 = persist.tile([128, NP, 2, HID], F8, tag="gg8")
        convbias = persist.tile([128, 3, 8], F32, tag="convbias")
        cvd = persist.tile([128, 9, 512], F16, tag="cvd")
        mgw = persist.tile([128, NP, 2, HID], F8, tag="mgw")
        mgb_p = persist.tile([128, 32], F32, tag="mgb")
        mgb = mgb_p[:, 0:NC]
        mixt = persist.tile([128, NC, HID], F16, tag="mixt")
        mixbias_p = persist.tile([128, 32], F32, tag="mixbias")
        h16 = [persist.tile([128, S], F16, tag=f"h{c}", name=f"h{c}")
               for c in range(NC)]
        hw16 = persist.tile([NH, S], F16, tag="hw16")

        def load_late_weights():
            # emitted after the first s-tile: keeps the startup DMA window
            # clear for x + gwT, which gate the first matmuls
            nc.scalar.dma_start(convbias, convbias_d[:, :, :])
            nc.scalar.dma_start(cvd, convdiag_d[:, :, :])
            nc.scalar.dma_start(mgw, mgw_d[:, :, :, :])
            nc.scalar.dma_start(mgb_p, mgb_d[:, :])
            nc.scalar.dma_start(mixt, mixt_d[:, :, :])
            nc.scalar.dma_start(mixbias_p, mixbias_d[:, :])

        # ---- phase A: router + GLU straight off the preloaded xT ----
        with (
            tc.tile_pool(name="xt", bufs=1) as p_xA,
            tc.tile_pool(name="psA", bufs=1, space="PSUM") as psA,
        ):
            # x arrives pre-transposed/pre-cast from the host; four big
            # chunks so the sync ring reaches line rate immediately
            xT = p_xA.tile([128, NC, S], F16, tag="xT")
            x8 = p_xA.tile([128, NP, 2, S], F8, tag="x8")
            # sync ring: fp16 x in four big token blocks.
            # scalar ring: fp8 x block 0, then the weights the first GLU
            # chains need (gate fp8, value per-kc slices), then fp8 x rest.
            nc.sync.dma_start(xT[:, :, 0:1024], x_d[:, :, 0:1024])
            # smallest-first on the scalar ring: the very first PE work (the
            # st=0 fp8 gate chain) needs only gg8 + the first 512 tokens of
            # x8, ~1MB total
            nc.scalar.dma_start(gg8, gg8_d[:, :, :, :])
            nc.scalar.dma_start(x8[:, :, :, 0:512], x8_d[:, :, :, 0:512])
            nc.scalar.dma_start(rwr, rwr_d[:, :, :])
            nc.scalar.dma_start(x8[:, :, :, 512:1024], x8_d[:, :, :, 512:1024])
            for sb in range(1, 4):
                nc.sync.dma_start(xT[:, :, sb * 1024:(sb + 1) * 1024],
                                  x_d[:, :, sb * 1024:(sb + 1) * 1024])
            for kc in range(NC):
                nc.scalar.dma_start(gvT[:, kc, :], gvT_d[:, kc, :])
            nc.scalar.dma_start(rb_p[:, 0:1], rb_d[:, :])
            for sb in range(1, 4):
                nc.scalar.dma_start(x8[:, :, :, sb * 1024:(sb + 1) * 1024],
                                    x8_d[:, :, :, sb * 1024:(sb + 1) * 1024])
            # PE warmup: keep the tensor engine busy through the HAM SHORT
            # window while the first x chunk streams in, so the real matmuls
            # start at the full 2.4 GHz clock instead of 1.2
            wup = p_xA.tile([128, ST], F16, tag="wup")
            nc.vector.memset(wup, 0.0)
            wps = psA.tile([NH, ST], F32, tag="rtr", bufs=2)
            for i in range(12):
                nc.tensor.matmul(wps[:, :], wup[:, 0:NH], wup[:, :],
                                 start=True, stop=True, skip_group_check=True)
            def router(st):
                s0 = st * ST
                pr = psA.tile([NH, ST], F32, tag="rtr", bufs=2)
                for kc in range(NC):
                    nc.tensor.matmul(pr[:, :], rwr[:, kc, 0:NH],
                                     xT[:, kc, s0:s0 + ST],
                                     start=(kc == 0), stop=(kc == NC - 1))
                nc.scalar.activation(hw16[:, s0:s0 + ST], pr[:, :], SIG,
                                     bias=rb[:, :], scale=1.0)

            for st in range(NST):
                s0 = st * ST
                if st == 3:
                    # not needed until phase B; keep the HBM window clear
                    load_late_weights()
                if st > 0:
                    # router first (head weights stay in SBUF); for tile 0
                    # it runs after the GLU so the first PE work only waits
                    # on the fp8 gate operands (smallest startup DMAs)
                    router(st)
                for oc in range(NC):
                    pg = psA.tile([128, ST], F32, tag="glu", bufs=6)
                    for q in range(NP):
                        nc.tensor.matmul(
                            pg[:, :],
                            gg8[:, q, :, oc * 128:(oc + 1) * 128],
                            x8[:, q, :, s0:s0 + ST],
                            start=(q == 0), stop=(q == NP - 1),
                            perf_mode=DR)
                    sg = p_sig.tile([128, ST], F32, tag="sg")
                    nc.scalar.activation(sg[:, :], pg[:, :], SIG)
                    pv = psA.tile([128, ST], F32, tag="glu", bufs=6)
                    for kc in range(NC):
                        nc.tensor.matmul(
                            pv[:, :],
                            gvT[:, kc, oc * 128:(oc + 1) * 128],
                            xT[:, kc, s0:s0 + ST],
                            start=(kc == 0), stop=(kc == NC - 1))
                    nc.vector.tensor_mul(h16[oc][:, s0:s0 + ST], pv[:, :], sg[:, :])
                if st == 0:
                    router(0)

        if dbg_d is not None and os.environ.get("KDBG") == "A":
            for c in range(NC):
                nc.sync.dma_start(dbg_d[c, :, :], h16[c][:, :])

        with tc.tile_pool(name="bcd", bufs=1) as p_bcd:
            # head-weight replica [128, NC, S]: chunk c rows 0:64 = head 2c,
            # 64:128 = head 2c+1. DMA can't partition-broadcast from SBUF,
            # so bounce hw16 through DRAM and broadcast on the way back.
            # (sync engine: idle during phase B — x is long loaded and the
            # first output stores are ~85us away)
            scr = p_dram.tile([NH, S], F16, tag="scr")
            nc.sync.dma_start(scr[:, :], hw16[:, :])
            hwr = p_bcd.tile([128, NC, S], F16, tag="hwr")
            for c in range(NC):
                nc.sync.dma_start(hwr[0:64, c, :],
                                  scr[2 * c, :].partition_broadcast(64))
                nc.sync.dma_start(hwr[64:128, c, :],
                                  scr[2 * c + 1, :].partition_broadcast(64))

            # ---- phase B: 3 conv stages, in-place over h16, 4-way packed.
            # Chunk-pair-outer so each pair finishes its three stages early:
            # its fused gate+fp8-cast (for the DoubleRow mix-gate) then
            # overlaps the next pair's conv instead of piling up at the B/C
            # boundary. The head-weight gating itself is DEFERRED out of the
            # conv evacs: h16 stays pre-gating; the gating is applied inside
            # the fused cast (mix-gate input) and folded into phase C's
            # sigmoid multiply. One PSUM pool spans B/C/D so phase C can
            # start while the last conv pair drains. ----
            h8 = p_bcd.tile([128, NP, 2, S], F8, tag="h8")
            # 2 PSUM banks for the mix-gate accumulators, claimed before the
            # conv pool so phase C's first chains don't wait on its drain
            from contextlib import ExitStack
            _mgstack = ExitStack()
            p_mg = _mgstack.enter_context(
                tc.tile_pool(name="psMG", bufs=1, space="PSUM"))
            with tc.tile_pool(name="psB", bufs=1, space="PSUM") as psB:
                for P in range(NP):
                    for j in range(3):
                        cA, cB = 2 * P, 2 * P + 1
                        groups = _conv_groups(j, P)
                        for st in reversed(range(NST)):
                            s0 = st * ST
                            pcs = {cA: psB.tile([128, ST], F32, tag="cvA",
                                                bufs=3, name=f"cvA{j}_{P}_{st}"),
                                   cB: psB.tile([128, ST], F32, tag="cvB",
                                                bufs=3, name=f"cvB{j}_{P}_{st}")}
                            # per group: list of (m, a, r0)
                            gtaps = []
                            for (c, ab, lp0, op0, tpos, d) in groups:
                                taps = []
                                for m in range(KT):
                                    off = m * d
                                    if off >= s0 + ST:
                                        continue
                                    a = max(0, off - s0)
                                    taps.append((m, a, s0 - off + a))
                                gtaps.append(taps)
                            nmax = max(len(t) for t in gtaps)
                            # round-robin the four blocks so LDWEIGHTS pulls
                            # ahead
                            for i in range(nmax):
                                for g, (c, ab, lp0, op0, tpos, d) in enumerate(groups):
                                    if i >= len(gtaps[g]):
                                        continue
                                    m, a, r0 = gtaps[g][i]
                                    nc.tensor.matmul(
                                        pcs[c][op0:op0 + 64, a:ST],
                                        cvd[lp0:lp0 + 64, 3 * j + P,
                                            ab * 256 + m * 64:ab * 256 + (m + 1) * 64],
                                        h16[c][lp0:lp0 + 64, r0:r0 + ST - a],
                                        start=(i == 0), stop=(i == len(gtaps[g]) - 1),
                                        tile_position=tpos)
                            if j < 2:
                                # evac split across engines so neither
                                # paces PE
                                nc.scalar.add(h16[cA][:, s0:s0 + ST],
                                              pcs[cA][:, :],
                                              convbias[:, j, cA:cA + 1])
                                nc.vector.tensor_scalar_add(
                                    h16[cB][:, s0:s0 + ST], pcs[cB][:, :],
                                    convbias[:, j, cB:cB + 1])
                            else:
                                # last stage folds the head-weight gating
                                # against the broadcast replica
                                nc.scalar.add(h16[cA][:, s0:s0 + ST],
                                              pcs[cA][:, :],
                                              convbias[:, 2, cA:cA + 1])
                                nc.vector.tensor_mul(h16[cA][:, s0:s0 + ST],
                                                     h16[cA][:, s0:s0 + ST],
                                                     hwr[:, cA, s0:s0 + ST])
                                nc.vector.scalar_tensor_tensor(
                                    h16[cB][:, s0:s0 + ST], pcs[cB][:, :],
                                    convbias[:, 2, cB:cB + 1],
                                    hwr[:, cB, s0:s0 + ST],
                                    op0=ADD, op1=MUL)
                    # fp8 casts for this pair (DoubleRow mix-gate input),
                    # back tokens first: stage-3 evacs complete descending
                    # and phases C/D consume descending. The LAST pair casts
                    # in quarters so phase C's first tile (st=7) is ready
                    # while this pair's stage-3 is still draining.
                    nq = 4 if P == NP - 1 else 2
                    qs = S // nq
                    for qi in reversed(range(nq)):
                        lo, hi = qi * qs, (qi + 1) * qs
                        nc.vector.tensor_copy(h8[:, P, 0, lo:hi],
                                              h16[cA][:, lo:hi])
                        nc.scalar.copy(h8[:, P, 1, lo:hi], h16[cB][:, lo:hi])

            if dbg_d is not None and os.environ.get("KDBG") == "B":
                for c in range(NC):
                    nc.sync.dma_start(dbg_d[c, :, :], h16[c][:, :])

            # ---- phase C: mix gate -> fp16 o16 (head gating folded
            # into the sigmoid multiply); D: final matmul with the
            # mixing weights stationary, output channel-major (host
            # transposes). C runs one s-tile ahead of D ----
            with tc.tile_pool(name="psC", bufs=1, space="PSUM") as psC:
                o16s = {}

                def stage_c(st):
                    s0 = st * ST
                    o16 = p_bcd.tile([128, NC, ST], F16, tag="o16", bufs=3,
                                     name="o16")
                    o16s[st] = o16
                    for oc in range(NC):
                        pm = p_mg.tile([128, ST], F32, tag="mg", bufs=2)
                        for q in range(NP):
                            nc.tensor.matmul(
                                pm[:, :],
                                mgw[:, q, :, oc * 128:(oc + 1) * 128],
                                h8[:, q, :, s0:s0 + ST],
                                start=(q == 0), stop=(q == NP - 1),
                                perf_mode=DR)
                        sg16 = p_sig.tile([128, ST], F16, tag="sg16", bufs=3)
                        nc.scalar.activation(sg16[:, :], pm[:, :], SIG,
                                             bias=mgb[:, oc:oc + 1], scale=1.0)
                        nc.vector.tensor_mul(o16[:, oc, :],
                                             h16[oc][:, s0:s0 + ST], sg16[:, :])

                def stage_d(st):
                    s0 = st * ST
                    o16 = o16s.pop(st)
                    for oc in range(NC):
                        pmx = psC.tile([128, ST], F32, tag="mx", bufs=3)
                        for kc in range(NC):
                            nc.tensor.matmul(
                                pmx[:, :],
                                mixt[:, kc, oc * 128:(oc + 1) * 128],
                                o16[:, kc, :],
                                start=(kc == 0), stop=(kc == NC - 1))
                        osb = p_out.tile([128, ST], F16, tag="osb")
                        nc.scalar.add(osb[:, :], pmx[:, :],
                                      mixbias_p[:, oc:oc + 1])
                        # all stores on the Sync ring: Scalar is the
                        # busy evac engine in this phase (sigmoids + bias
                        # adds) while Sync idles at ~13%
                        nc.sync.dma_start(out_d[oc * 128:(oc + 1) * 128,
                                                s0:s0 + ST], osb[:, :])

                # C runs two s-tiles ahead of D
                stage_c(NST - 1)
                stage_c(NST - 2)
                for st in reversed(range(NST)):
                    stage_d(st)
                    if st - 2 >= 0:
                        stage_c(st - 2)


def _prep_weights(gate_w, conv_w, conv_b, router_w, router_b,
                  mix_gate_w, mix_gate_b, mixing_w, mixing_b):
    f = np.float32

    # GLU output-channel permutation: odd chunks written half-swapped
    perm = np.arange(HID)
    for c in range(1, NC, 2):
        lo = perm[c * 128:c * 128 + 64].copy()
        perm[c * 128:c * 128 + 64] = perm[c * 128 + 64:(c + 1) * 128]
        perm[c * 128 + 64:(c + 1) * 128] = lo
    # value half fp16 [128, kc, HID]; gate half fp8 DoubleRow-interleaved
    # [128, pair, 2, HID] (K-interleave matches xb8's channel layout)
    gvT = np.ascontiguousarray(
        gate_w[perm].T.reshape(NC, 128, HID).transpose(1, 0, 2),
        dtype=np.float16)
    gg8 = np.ascontiguousarray(
        gate_w[HID + perm].T.reshape(NP, 2, 128, HID).transpose(2, 0, 1, 3)
    ).astype(mybir.dt.np(F8))

    rwr = np.zeros((128, NC, 64), dtype=np.float16)
    rwr[:, :, 0:NH] = router_w.T.reshape(NC, 128, NH).transpose(1, 0, 2)
    rb = np.ascontiguousarray(router_b.reshape(NH, 1), dtype=f)

    # conv tap diagonals [128, 9(jP), 512(ab,m,64)], residual folded into m=0
    cd = np.zeros((128, 9, 512), dtype=np.float16)
    ar = np.arange(HD)
    for j in range(3):
        for P in range(NP):
            for (c, ab, lp0, op0, tpos, d) in _conv_groups(j, P):
                # which head streams through rows lp0..lp0+64
                if ab == 0:
                    head = 2 * c + (1 if lp0 == 64 else 0)
                elif LIN[j]:
                    head = 2 * c + 1 if lp0 == 0 else 2 * c
                else:
                    head = 2 * c if lp0 == 0 else 2 * c + 1
                for m in range(KT):
                    w = conv_w[head, j, :, KT - 1 - m].astype(f)
                    if m == 0:
                        w = w + 1.0
                    cd[lp0 + ar, 3 * j + P, ab * 256 + m * 64 + ar] = \
                        w.astype(np.float16)
    convdiag = np.ascontiguousarray(cd)

    # conv bias per (stage, chunk) under the OUTPUT layout of that stage
    cb = np.zeros((128, 3, 8), dtype=f)
    for j in range(3):
        for c in range(NC):
            lout = (1 - LIN[j]) if (c % 2 == 1) else 0
            if lout == 0:
                cb[0:64, j, c] = conv_b[2 * c, j]
                cb[64:128, j, c] = conv_b[2 * c + 1, j]
            else:
                cb[0:64, j, c] = conv_b[2 * c + 1, j]
                cb[64:128, j, c] = conv_b[2 * c, j]
    convbias = np.ascontiguousarray(cb)

    # mix-gate weights fp8 e4m3, DoubleRow-interleaved [128, pair, 2, HID]:
    # [p, q, i, o] = W^T[(2q+i)*128 + p, o]
    mgw = np.ascontiguousarray(
        mix_gate_w.T.reshape(NP, 2, 128, HID).transpose(2, 0, 1, 3)
    ).astype(mybir.dt.np(F8))
    mgb = np.zeros((128, 32), dtype=f)
    mgb[:, 0:NC] = mix_gate_b.reshape(NC, 128).T
    mixt = np.ascontiguousarray(
        mixing_w.T.astype(np.float16).reshape(NC, 128, HID).transpose(1, 0, 2))
    mixbias = np.zeros((128, 32), dtype=f)
    mixbias[:, 0:NC] = mixing_b.reshape(NC, 128).T

    return {"gvT16": gvT, "gg8": gg8, "rwr16": rwr, "rb": rb,
            "convdiag": convdiag, "convbias": convbias,
            "mgw8": mgw, "mgb": mgb,
            "mixt16": mixt, "mixbias": mixbias}


_CACHE = {}


def _run(inputs, trace=False, tmpdir=None):
    if "nc" not in _CACHE:
        _CACHE["nc"] = build_bass()
    nc = _CACHE["nc"]

    w = _prep_weights(
        np.asarray(inputs["gate_w"]), np.asarray(inputs["conv_w"]),
        np.asarray(inputs["conv_b"]), np.asarray(inputs["router_w"]),
        np.asarray(inputs["router_b"]), np.asarray(inputs["mix_gate_w"]),
        np.asarray(inputs["mix_gate_b"]), np.asarray(inputs["mixing_w"]),
        np.asarray(inputs["mixing_b"]))
    # ship x pre-transposed + pre-cast (same rounding the on-chip DVE cast
    # would apply), channel-major [128, chunk, S]
    x = np.asarray(inputs["x"])
    f8 = mybir.dt.np(F8)
    in_maps = []
    for b in range(B):
        xt = x[b].T.astype(np.float16)  # [HID, S]
        in_maps.append(dict(
            w,
            xbT=np.ascontiguousarray(
                xt.reshape(NC, 128, S).transpose(1, 0, 2)),
            xb8=np.ascontiguousarray(
                xt.reshape(NP, 2, 128, S).transpose(2, 0, 1, 3)).astype(f8),
        ))
    res = run_bass_kernel_spmd(nc, in_maps, core_ids=list(range(B)),
                               trace=trace, tmpdir=tmpdir)
    # device output is channel-major [HID, S]; transpose on unshard
    out = np.stack([res.results[b]["out"].T for b in range(B)],
                   axis=0).astype(np.float32)
    return out, res


def kernel(**inputs):
    out, _ = _run(inputs, trace=False)
    return out


if __name__ == "__main__":
    nc = build_bass()
    print("built ok; instructions:", len(nc.inst_map))


# revision 75
# speedup vs baseline: 1.0061x; 1.0061x over previous
"""Trainium2 Bass kernel for nn_MultiHeadDilatedState.

Sharding: data-parallel over batch (B=8 -> 8 cores, one sequence per core).
Weights replicated. Per-core dataflow is channel-major [768, 4096], with the
whole h pipeline held in fp16 SBUF:

  x ships from the host BOTH pre-transposed fp16 [128, chunk, S] and as an
  fp8-e4m3 DoubleRow-interleaved copy [128, pair, 2, S]. A short warmup
  matmul spin keeps the PE's HAM clock warm while the first x block lands.
  Phase A: router fp16; GLU gate half as fp8 DoubleRow matmuls (K=256 per
  pass, sigmoid damps the quantization error), value half fp16; PSUM
  accumulates fp32. The router head-weights are broadcast to a
  [128, NC, S] per-channel replica via a DRAM-roundtrip DMA
  (partition-broadcast access pattern) for the stage-3 gating.
  Phase B (convs): per-head depthwise dilated conv = fp16 diagonal matmuls
  with column-shifted rhs APs; TWO chunks (4 heads) run concurrently using
  all four (row,col) 64x64 blocks of the PE array via tile_position; the
  residual folds into the tap-0 diag (1+w). Chunk-pair-outer/stage-inner
  ordering lets each pair's fp8 h casts (mix-gate input) overlap the next
  pair's conv. Stage-3 evacs fuse the head-weight gating (Scalar/Vector
  split).
  Phase C: mix-gate as fp8 DoubleRow matmuls off the cast h8; phase D
  keeps the mixing weights stationary and emits the output channel-major
  [768, S] (host transposes on unshard); C runs two s-tiles ahead of D,
  both descending to chase the conv's completion order.
"""

import os
import numpy as np

import concourse.bass as bass
import concourse.bacc as bacc
import concourse.mybir as mybir
import concourse.tile as tile
from concourse.bass_utils import run_bass_kernel_spmd

B, S, HID = 8, 4096, 768
NH, HD, KT = 12, 64, 4  # heads, head_dim, kernel taps
NC = 6                  # 768 / 128 channel chunks
NP = 3                  # chunk pairs
ST = 512                # token tile
NST = S // ST           # 8
F32 = mybir.dt.float32
F16 = mybir.dt.float16
F8 = mybir.dt.float8e4
DR = mybir.MatmulPerfMode.DoubleRow
SIG = mybir.ActivationFunctionType.Sigmoid
ADD = mybir.AluOpType.add
MUL = mybir.AluOpType.mult

DILATIONS = [(1, 2, 4), (1, 1, 1), (4, 8, 16), (8, 16, 32), (32, 64, 128),
             (64, 128, 256), (256, 512, 1024), (1, 100, 200), (1, 500, 1000),
             (1, 1024, 2048), (3, 9, 27), (5, 25, 125)]

# odd-chunk storage layout entering stage j: 1 = halves swapped. Each conv
# stage flips it (cross tile_position blocks exchange halves); GLU writes
# odd chunks swapped so the final stage lands back on normal order.
LIN = [1, 0, 1]


def build_bass():
    nc = bacc.Bacc()

    x_d = nc.dram_tensor("xbT", [128, NC, S], F16, kind="ExternalInput")
    x8_d = nc.dram_tensor("xb8", [128, NP, 2, S], F8, kind="ExternalInput")
    gvT_d = nc.dram_tensor("gvT16", [128, NC, HID], F16, kind="ExternalInput")
    gg8_d = nc.dram_tensor("gg8", [128, NP, 2, HID], F8, kind="ExternalInput")
    rwr_d = nc.dram_tensor("rwr16", [128, NC, 64], F16, kind="ExternalInput")
    rb_d = nc.dram_tensor("rb", [NH, 1], F32, kind="ExternalInput")
    convdiag_d = nc.dram_tensor("convdiag", [128, 9, 512], F16, kind="ExternalInput")
    convbias_d = nc.dram_tensor("convbias", [128, 3, 8], F32, kind="ExternalInput")
    mgw_d = nc.dram_tensor("mgw8", [128, NP, 2, HID], F8, kind="ExternalInput")
    mgb_d = nc.dram_tensor("mgb", [128, 32], F32, kind="ExternalInput")
    mixt_d = nc.dram_tensor("mixt16", [128, NC, HID], F16, kind="ExternalInput")
    mixbias_d = nc.dram_tensor("mixbias", [128, 32], F32, kind="ExternalInput")
    out_d = nc.dram_tensor("out", [HID, S], F16, kind="ExternalOutput")
    dbg_d = nc.dram_tensor("dbg", [NC, 128, S], F16, kind="ExternalOutput") if os.environ.get("KDBG") else None

    with tile.TileContext(nc) as tc:
        _body(tc, x_d, x8_d, gvT_d, gg8_d, rwr_d, rb_d, convdiag_d,
              convbias_d, mgw_d, mgb_d, mixt_d, mixbias_d, out_d, dbg_d)
    nc.finalize()
    return nc


def _conv_groups(j, P):
    """The four 64x64 PE-array blocks for chunk pair (2P, 2P+1) at stage j.

    Returns (chunk, ab, lhs_p0, out_p0, tile_position, dilation) per block.
    lhs_p0: partition base of the stationary diag (= rhs row group).
    out_p0: psum partition base (= col group).
    """
    cA, cB = 2 * P, 2 * P + 1
    hT = 2 * cB + 1 if LIN[j] else 2 * cB          # cB data on partitions 0:64
    hU = 2 * cB if LIN[j] else 2 * cB + 1          # cB data on partitions 64:128
    return [
        (cA, 0, 0, 0, (0, 0), DILATIONS[2 * cA][j]),
        (cA, 0, 64, 64, (64, 64), DILATIONS[2 * cA + 1][j]),
        (cB, 1, 0, 64, (0, 64), DILATIONS[hT][j]),
        (cB, 1, 64, 0, (64, 0), DILATIONS[hU][j]),
    ]


def _body(tc, x_d, x8_d, gvT_d, gg8_d, rwr_d, rb_d, convdiag_d, convbias_d,
          mgw_d, mgb_d, mixt_d, mixbias_d, out_d, dbg_d=None):
    nc = tc.nc

    with (
        tc.tile_pool(name="persist", bufs=1) as persist,
        tc.tile_pool(name="sig", bufs=4) as p_sig,
        tc.tile_pool(name="outsb", bufs=4) as p_out,
        tc.tile_pool(name="dscr", bufs=1, space="DRAM") as p_dram,
    ):
        # ---- persistent weights. Weight DMAs go on the Activation DGE
        # ring so the x loads (Sync ring) aren't stuck behind them;
        # earliest-needed weights first. ----
        rwr = persist.tile([128, NC, 64], F16, tag="rwr")
        rb_p = persist.tile([NH, 32], F32, tag="rb")
        rb = rb_p[:, 0:1]
        gvT = persist.tile([128, NC, HID], F16, tag="gvT")
        gg8 = persist.tile([128, NP, 2, HID], F8, tag="gg8")
        convbias = persist.tile([128, 3, 8], F32, tag="convbias")
        cvd = persist.tile([128, 9, 512], F16, tag="cvd")
        mgw = persist.tile([128, NP, 2, HID], F8, tag="mgw")
        mgb_p = persist.tile([128, 32], F32, tag="mgb")
        mgb = mgb_p[:, 0:NC]
        mixt = persist.tile([128, NC, HID], F16, tag="mixt")
        mixbias_p = persist.tile([128, 32], F32, tag="mixbias")
        h16 = [persist.tile([128, S], F16, tag=f"h{c}", name=f"h{c}")
               for c in range(NC)]
        hw16 = persist.tile([NH, S], F16, tag="hw16")

        def load_late_weights():
            # emitted after the first s-tile: keeps the startup DMA window
            # clear for x + gwT, which gate the first matmuls
            nc.scalar.dma_start(convbias, convbias_d[:, :, :])
            nc.scalar.dma_start(cvd, convdiag_d[:, :, :])
            nc.scalar.dma_start(mgw, mgw_d[:, :, :, :])
            nc.scalar.dma_start(mgb_p, mgb_d[:, :])
            nc.scalar.dma_start(mixt, mixt_d[:, :, :])
            nc.scalar.dma_start(mixbias_p, mixbias_d[:, :])

        # ---- phase A: router + GLU straight off the preloaded xT ----
        with (
            tc.tile_pool(name="xt", bufs=1) as p_xA,
            tc.tile_pool(name="psA", bufs=1, space="PSUM") as psA,
        ):
            # x arrives pre-transposed/pre-cast from the host; four big
            # chunks so the sync ring reaches line rate immediately
            xT = p_xA.tile([128, NC, S], F16, tag="xT")
            x8 = p_xA.tile([128, NP, 2, S], F8, tag="x8")
            # sync ring: fp16 x in four big token blocks.
            # scalar ring: fp8 x block 0, then the weights the first GLU
            # chains need (gate fp8, value per-kc slices), then fp8 x rest.
            nc.sync.dma_start(xT[:, :, 0:1024], x_d[:, :, 0:1024])
            # smallest-first on the scalar ring: the very first PE work (the
            # st=0 fp8 gate chain) needs only gg8 + the first 512 tokens of
            # x8, ~1MB total
            nc.scalar.dma_start(gg8, gg8_d[:, :, :, :])
            nc.scalar.dma_start(x8[:, :, :, 0:512], x8_d[:, :, :, 0:512])
            nc.scalar.dma_start(rwr, rwr_d[:, :, :])
            nc.scalar.dma_start(x8[:, :, :, 512:1024], x8_d[:, :, :, 512:1024])
            for sb in range(1, 4):
                nc.sync.dma_start(xT[:, :, sb * 1024:(sb + 1) * 1024],
                                  x_d[:, :, sb * 1024:(sb + 1) * 1024])
            for kc in range(NC):
                nc.scalar.dma_start(gvT[:, kc, :], gvT_d[:, kc, :])
            nc.scalar.dma_start(rb_p[:, 0:1], rb_d[:, :])
            for sb in range(1, 4):
                nc.scalar.dma_start(x8[:, :, :, sb * 1024:(sb + 1) * 1024],
                                    x8_d[:, :, :, sb * 1024:(sb + 1) * 1024])
            # PE warmup: keep the tensor engine busy through the HAM SHORT
            # window while the first x chunk streams in, so the real matmuls
            # start at the full 2.4 GHz clock instead of 1.2
            wup = p_xA.tile([128, ST], F16, tag="wup")
            nc.vector.memset(wup, 0.0)
            wps = psA.tile([NH, ST], F32, tag="rtr", bufs=2)
            for i in range(12):
                nc.tensor.matmul(wps[:, :], wup[:, 0:NH], wup[:, :],
                                 start=True, stop=True, skip_group_check=True)
            def router(st):
                s0 = st * ST
                pr = psA.tile([NH, ST], F32, tag="rtr", bufs=2)
                for kc in range(NC):
                    nc.tensor.matmul(pr[:, :], rwr[:, kc, 0:NH],
                                     xT[:, kc, s0:s0 + ST],
                                     start=(kc == 0), stop=(kc == NC - 1))
                nc.scalar.activation(hw16[:, s0:s0 + ST], pr[:, :], SIG,
                                     bias=rb[:, :], scale=1.0)

            for st in range(NST):
                s0 = st * ST
                if st == 3:
                    # not needed until phase B; keep the HBM window clear
                    load_late_weights()
                if st > 0:
                    # router first (head weights stay in SBUF); for tile 0
                    # it runs after the GLU so the first PE work only waits
                    # on the fp8 gate operands (smallest startup DMAs)
                    router(st)
                for oc in range(NC):
                    pg = psA.tile([128, ST], F32, tag="glu", bufs=6)
                    for q in range(NP):
                        nc.tensor.matmul(
                            pg[:, :],
                            gg8[:, q, :, oc * 128:(oc + 1) * 128],
                            x8[:, q, :, s0:s0 + ST],
                            start=(q == 0), stop=(q == NP - 1),
                            perf_mode=DR)
                    sg = p_sig.tile([128, ST], F32, tag="sg")
                    nc.scalar.activation(sg[:, :], pg[:, :], SIG)
                    pv = psA.tile([128, ST], F32, tag="glu", bufs=6)
                    for kc in range(NC):
                        nc.tensor.matmul(
                            pv[:, :],
                            gvT[:, kc, oc * 128:(oc + 1) * 128],
                            xT[:, kc, s0:s0 + ST],
                            start=(kc == 0), stop=(kc == NC - 1))
                    nc.vector.tensor_mul(h16[oc][:, s0:s0 + ST], pv[:, :], sg[:, :])
                if st == 0:
                    router(0)

        if dbg_d is not None and os.environ.get("KDBG") == "A":
            for c in range(NC):
                nc.sync.dma_start(dbg_d[c, :, :], h16[c][:, :])

        with tc.tile_pool(name="bcd", bufs=1) as p_bcd:
            # head-weight replica [128, NC, S]: chunk c rows 0:64 = head 2c,
            # 64:128 = head 2c+1. DMA can't partition-broadcast from SBUF,
            # so bounce hw16 through DRAM and broadcast on the way back.
            # (sync engine: idle during phase B — x is long loaded and the
            # first output stores are ~85us away)
            scr = p_dram.tile([NH, S], F16, tag="scr")
            nc.sync.dma_start(scr[:, :], hw16[:, :])
            hwr = p_bcd.tile([128, NC, S], F16, tag="hwr")
            for c in range(NC):
                nc.sync.dma_start(hwr[0:64, c, :],
                                  scr[2 * c, :].partition_broadcast(64))
                nc.sync.dma_start(hwr[64:128, c, :],
                                  scr[2 * c + 1, :].partition_broadcast(64))

            # ---- phase B: 3 conv stages, in-place over h16, 4-way packed.
            # Chunk-pair-outer so each pair finishes its three stages early:
            # its fused gate+fp8-cast (for the DoubleRow mix-gate) then
            # overlaps the next pair's conv instead of piling up at the B/C
            # boundary. The head-weight gating itself is DEFERRED out of the
            # conv evacs: h16 stays pre-gating; the gating is applied inside
            # the fused cast (mix-gate input) and folded into phase C's
            # sigmoid multiply. One PSUM pool spans B/C/D so phase C can
            # start while the last conv pair drains. ----
            h8 = p_bcd.tile([128, NP, 2, S], F8, tag="h8")
            # 2 PSUM banks for the mix-gate accumulators, claimed before the
            # conv pool so phase C's first chains don't wait on its drain
            from contextlib import ExitStack
            _mgstack = ExitStack()
            p_mg = _mgstack.enter_context(
                tc.tile_pool(name="psMG", bufs=1, space="PSUM"))
            with tc.tile_pool(name="psB", bufs=1, space="PSUM") as psB:
                for P in range(NP):
                    for j in range(3):
                        cA, cB = 2 * P, 2 * P + 1
                        groups = _conv_groups(j, P)
                        for st in reversed(range(NST)):
                            s0 = st * ST
                            pcs = {cA: psB.tile([128, ST], F32, tag="cvA",
                                                bufs=3, name=f"cvA{j}_{P}_{st}"),
                                   cB: psB.tile([128, ST], F32, tag="cvB",
                                                bufs=3, name=f"cvB{j}_{P}_{st}")}
                            # per group: list of (m, a, r0)
                            gtaps = []
                            for (c, ab, lp0, op0, tpos, d) in groups:
                                taps = []
                                for m in range(KT):
                                    off = m * d
                                    if off >= s0 + ST:
                                        continue
                                    a = max(0, off - s0)
                                    taps.append((m, a, s0 - off + a))
                                gtaps.append(taps)
                            nmax = max(len(t) for t in gtaps)
                            # round-robin the four blocks so LDWEIGHTS pulls
                            # ahead
                            for i in range(nmax):
                                for g, (c, ab, lp0, op0, tpos, d) in enumerate(groups):
                                    if i >= len(gtaps[g]):
                                        continue
                                    m, a, r0 = gtaps[g][i]
                                    nc.tensor.matmul(
                                        pcs[c][op0:op0 + 64, a:ST],
                                        cvd[lp0:lp0 + 64, 3 * j + P,
                                            ab * 256 + m * 64:ab * 256 + (m + 1) * 64],
                                        h16[c][lp0:lp0 + 64, r0:r0 + ST - a],
                                        start=(i == 0), stop=(i == len(gtaps[g]) - 1),
                                        tile_position=tpos)
                            if j < 2:
                                # evac split across engines so neither
                                # paces PE
                                nc.scalar.add(h16[cA][:, s0:s0 + ST],
                                              pcs[cA][:, :],
                                              convbias[:, j, cA:cA + 1])
                                nc.vector.tensor_scalar_add(
                                    h16[cB][:, s0:s0 + ST], pcs[cB][:, :],
                                    convbias[:, j, cB:cB + 1])
                            else:
                                # last stage folds the head-weight gating
                                # against the broadcast replica
                                nc.scalar.add(h16[cA][:, s0:s0 + ST],
                                              pcs[cA][:, :],
                                              convbias[:, 2, cA:cA + 1])
                                nc.vector.tensor_mul(h16[cA][:, s0:s0 + ST],
                                                     h16[cA][:, s0:s0 + ST],
                                                     hwr[:, cA, s0:s0 + ST])
                                nc.vector.scalar_tensor_tensor(
                                    h16[cB][:, s0:s0 + ST], pcs[cB][:, :],
                                    convbias[:, 2, cB:cB + 1],
                                    hwr[:, cB, s0:s0 + ST],
                                    op0=ADD, op1=MUL)
                    # fp8 casts for this pair (DoubleRow mix-gate input),
                    # back tokens first: stage-3 evacs complete descending
                    # and phases C/D consume descending. The LAST pair casts
                    # in quarters so phase C's first tile (st=7) is ready
                    # while this pair's stage-3 is still draining.
                    nq = 4 if P == NP - 1 else 2
                    qs = S // nq
                    for qi in reversed(range(nq)):
                        lo, hi = qi * qs, (qi + 1) * qs
                        nc.vector.tensor_copy(h8[:, P, 0, lo:hi],
                                              h16[cA][:, lo:hi])
                        nc.scalar.copy(h8[:, P, 1, lo:hi], h16[cB][:, lo:hi])

            if dbg_d is not None and os.environ.get("KDBG") == "B":
                for c in range(NC):
                    nc.sync.dma_start(dbg_d[c, :, :], h16[c][:, :])

            # ---- phase C: mix gate -> fp16 o16 (head gating folded
            # into the sigmoid multiply); D: final matmul with the
            # mixing weights stationary, output channel-major (host
            # transposes). C runs one s-tile ahead of D ----
            with tc.tile_pool(name="psC", bufs=1, space="PSUM") as psC:
                o16s = {}

                def stage_c(st):
                    s0 = st * ST
                    o16 = p_bcd.tile([128, NC, ST], F16, tag="o16", bufs=3,
                                     name="o16")
                    o16s[st] = o16
                    for oc in range(NC):
                        pm = p_mg.tile([128, ST], F32, tag="mg", bufs=2)
                        for q in range(NP):
                            nc.tensor.matmul(
                                pm[:, :],
                                mgw[:, q, :, oc * 128:(oc + 1) * 128],
                                h8[:, q, :, s0:s0 + ST],
                                start=(q == 0), stop=(q == NP - 1),
                                perf_mode=DR)
                        sg16 = p_sig.tile([128, ST], F16, tag="sg16", bufs=3)
                        nc.scalar.activation(sg16[:, :], pm[:, :], SIG,
                                             bias=mgb[:, oc:oc + 1], scale=1.0)
                        nc.vector.tensor_mul(o16[:, oc, :],
                                             h16[oc][:, s0:s0 + ST], sg16[:, :])

                def stage_d(st):
                    s0 = st * ST
                    o16 = o16s.pop(st)
                    for oc in range(NC):
                        pmx = psC.tile([128, ST], F32, tag="mx", bufs=3)
                        for kc in range(NC):
                            nc.tensor.matmul(
                                pmx[:, :],
                                mixt[:, kc, oc * 128:(oc + 1) * 128],
                                o16[:, kc, :],
                                start=(kc == 0), stop=(kc == NC - 1))
                        osb = p_out.tile([128, ST], F16, tag="osb")
                        nc.scalar.add(osb[:, :], pmx[:, :],
                                      mixbias_p[:, oc:oc + 1])
                        eng = nc.sync if oc % 2 == 0 else nc.scalar
                        eng.dma_start(out_d[oc * 128:(oc + 1) * 128,
                                            s0:s0 + ST], osb[:, :])

                # C runs two s-tiles ahead of D
                stage_c(NST - 1)
                stage_c(NST - 2)
                for st in reversed(range(NST)):
                    stage_d(st)
                    if st - 2 >= 0:
                        stage_c(st - 2)


def _prep_weights(gate_w, conv_w, conv_b, router_w, router_b,
                  mix_gate_w, mix_gate_b, mixing_w, mixing_b):
    f = np.float32

    # GLU output-channel permutation: odd chunks written half-swapped
    perm = np.arange(HID)
    for c in range(1, NC, 2):
        lo = perm[c * 128:c * 128 + 64].copy()
        perm[c * 128:c * 128 + 64] = perm[c * 128 + 64:(c + 1) * 128]
        perm[c * 128 + 64:(c + 1) * 128] = lo
    # value half fp16 [128, kc, HID]; gate half fp8 DoubleRow-interleaved
    # [128, pair, 2, HID] (K-interleave matches xb8's channel layout)
    gvT = np.ascontiguousarray(
        gate_w[perm].T.reshape(NC, 128, HID).transpose(1, 0, 2),
        dtype=np.float16)
    gg8 = np.ascontiguousarray(
        gate_w[HID + perm].T.reshape(NP, 2, 128, HID).transpose(2, 0, 1, 3)
    ).astype(mybir.dt.np(F8))

    rwr = np.zeros((128, NC, 64), dtype=np.float16)
    rwr[:, :, 0:NH] = router_w.T.reshape(NC, 128, NH).transpose(1, 0, 2)
    rb = np.ascontiguousarray(router_b.reshape(NH, 1), dtype=f)

    # conv tap diagonals [128, 9(jP), 512(ab,m,64)], residual folded into m=0
    cd = np.zeros((128, 9, 512), dtype=np.float16)
    ar = np.arange(HD)
    for j in range(3):
        for P in range(NP):
            for (c, ab, lp0, op0, tpos, d) in _conv_groups(j, P):
                # which head streams through rows lp0..lp0+64
                if ab == 0:
                    head = 2 * c + (1 if lp0 == 64 else 0)
                elif LIN[j]:
                    head = 2 * c + 1 if lp0 == 0 else 2 * c
                else:
                    head = 2 * c if lp0 == 0 else 2 * c + 1
                for m in range(KT):
                    w = conv_w[head, j, :, KT - 1 - m].astype(f)
                    if m == 0:
                        w = w + 1.0
                    cd[lp0 + ar, 3 * j + P, ab * 256 + m * 64 + ar] = \
                        w.astype(np.float16)
    convdiag = np.ascontiguousarray(cd)

    # conv bias per (stage, chunk) under the OUTPUT layout of that stage
    cb = np.zeros((128, 3, 8), dtype=f)
    for j in range(3):
        for c in range(NC):
            lout = (1 - LIN[j]) if (c % 2 == 1) else 0
            if lout == 0:
                cb[0:64, j, c] = conv_b[2 * c, j]
                cb[64:128, j, c] = conv_b[2 * c + 1, j]
            else:
                cb[0:64, j, c] = conv_b[2 * c + 1, j]
                cb[64:128, j, c] = conv_b[2 * c, j]
    convbias = np.ascontiguousarray(cb)

    # mix-gate weights fp8 e4m3, DoubleRow-interleaved [128, pair, 2, HID]:
    # [p, q, i, o] = W^T[(2q+i)*128 + p, o]
    mgw = np.ascontiguousarray(
        mix_gate_w.T.reshape(NP, 2, 128, HID).transpose(2, 0, 1, 3)
    ).astype(mybir.dt.np(F8))
    mgb = np.zeros((128, 32), dtype=f)
    mgb[:, 0:NC] = mix_gate_b.reshape(NC, 128).T
    mixt = np.ascontiguousarray(
        mixing_w.T.astype(np.float16).reshape(NC, 128, HID).transpose(1, 0, 2))
    mixbias = np.zeros((128, 32), dtype=f)
    mixbias[:, 0:NC] = mixing_b.reshape(NC, 128).T

    return {"gvT16": gvT, "gg8": gg8, "rwr16": rwr, "rb": rb,
            "convdiag": convdiag, "convbias": convbias,
            "mgw8": mgw, "mgb": mgb,
            "mixt16": mixt, "mixbias": mixbias}


_CACHE = {}


def _run(inputs, trace=False, tmpdir=None):
    if "nc" not in _CACHE:
        _CACHE["nc"] = build_bass()
    nc = _CACHE["nc"]

    w = _prep_weights(
        np.asarray(inputs["gate_w"]), np.asarray(inputs["conv_w"]),
        np.asarray(inputs["conv_b"]), np.asarray(inputs["router_w"]),
        np.asarray(inputs["router_b"]), np.asarray(inputs["mix_gate_w"]),
        np.asarray(inputs["mix_gate_b"]), np.asarray(inputs["mixing_w"]),
        np.asarray(inputs["mixing_b"]))
    # ship x pre-transposed + pre-cast (same rounding the on-chip DVE cast
    # would apply), channel-major [128, chunk, S]
    x = np.asarray(inputs["x"])
    f8 = mybir.dt.np(F8)
    in_maps = []
    for b in range(B):
        xt = x[b].T.astype(np.float16)  # [HID, S]
        in_maps.append(dict(
            w,
            xbT=np.ascontiguousarray(
                xt.reshape(NC, 128, S).transpose(1, 0, 2)),
            xb8=np.ascontiguousarray(
                xt.reshape(NP, 2, 128, S).transpose(2, 0, 1, 3)).astype(f8),
        ))
    res = run_bass_kernel_spmd(nc, in_maps, core_ids=list(range(B)),
                               trace=trace, tmpdir=tmpdir)
    # device output is channel-major [HID, S]; transpose on unshard
    out = np.stack([res.results[b]["out"].T for b in range(B)],
                   axis=0).astype(np.float32)
    return out, res


def kernel(**inputs):
    out, _ = _run(inputs, trace=False)
    return out


if __name__ == "__main__":
    nc = build_bass()
    print("built ok; instructions:", len(nc.inst_map))
